# revision 41
# baseline (speedup 1.0000x reference)
"""Trainium2 Bass kernel for a feature-space attention head.

Reference computation (per batch b, with T=4096, E=1024, D=64):
    Q = x @ Wq; K = x @ Wk; V = x @ Wv            # (T,E)@(E,D) -> (T,D)
    R = (K^T @ Q) / sqrt(E)                        # (D,D) feature-space scores
    R = where(strictly_lower, -inf, R); R = softmax(R, axis=-1)
    out = V @ R                                    # (T,D)

Sharding: data-parallel over batch B=8 across the 8 NeuronCores (one batch
per core, no collectives).

Per-core device pipeline (bf16 operands, fp32 PSUM accumulation):
  - host pre-casts to bf16 and packs x TILE-CONTIGUOUSLY: xh[p, (i*8+c)*128+u]
    = x[i*128+u, c*128+p], so each DMA piece of ntl t-tiles moves with 128
    descriptors of ntl*2KB (small descriptors starve the 16 DMA engines on
    descriptor generation; 2KB+ lines reach the full ~360 GB/s).
  - w3 = [Wq/32 | Wk | Wv] packed partition-major rides in the same DMA as
    the first x piece (one config+DGE+sem latency on the critical head
    instead of two); all pieces stream on the sync queue in consumption
    order (per-queue FIFO = need order).
  - warmup matmuls on a zeroed scratch tile run during the DMA head so the
    PE p-state ramp (0.65 -> 2.4 GHz over ~3us of continuous work) is done
    before real work. Every PE instruction costs ~83ns steady-state, so the
    design minimizes PE instruction count.
  - per 128-row t-tile: one joint QKV matmul pass (stationary = x^T chunk,
    moving = w3 [128,192]) -> QKV natural in PSUM; two copies per tile
    ([Q|K] half and V half on alternating ACT/DVE) so consecutive V tiles
    land adjacent in a [128, 128] pair tile.
  - per 8-tile batch: 8x R += K^T Q into a persistent PSUM bank; V pair
    tiles transpose in ONE [128,128] PE transpose each (V_even^T on
    partitions 0-63, V_odd^T on 64-127), halving transpose instructions;
    one DVE copy per batch into persistent vT2 [128, 2048].
  - the causal mask is added as a final R accumulation matmul (I64^T @ mask)
    instead of a DVE add, shortening the softmax tail; softmax runs on R in
    fp32 without max-shift (scores bounded), P is replicated to partitions
    64-127 with two tiny PE transposes so odd-tile O matmuls can use the
    vT2 upper half directly.
  - O = V @ P in 4 groups of 8 chunks (even tiles first, then odd tiles);
    output staged bf16 (host casts back to f32); group DMAs leave on
    different queues so their config latencies overlap.
"""

import os
import sys

import numpy as np

for _p in ("/opt/trn_rl_repo", "/root/.axon_site/_ro/trn_rl_repo"):
    if os.path.isdir(_p) and _p not in sys.path:
        sys.path.append(_p)

import ml_dtypes  # noqa: E402

import concourse.bass as bass  # noqa: E402
import concourse.tile as tile  # noqa: E402
from concourse import bacc, mybir  # noqa: E402
from concourse.bass_utils import run_bass_kernel_spmd  # noqa: E402

B, T, E, D = 8, 4096, 1024, 64
N_CORES = 8
M3 = 3 * D                # 192: packed [Q|K|V] output columns
ECH = E // 128            # 8 e-chunks
NT = T // 128             # 32 t-tiles
TW = ECH * 128            # 1024: columns per t-tile in the packed x layout

F32 = mybir.dt.float32
BF16 = mybir.dt.bfloat16
AX = mybir.AxisListType
AF = mybir.ActivationFunctionType

_COMPILED = None

# x-piece schedule (start tile, #tiles, queue): piece 0 also carries w3
# (one config+DGE+sem on the critical head instead of two). Everything
# stays on the sync queue in consumption order: a second queue splits the
# DMA engines and delays the early pieces the PE is waiting on (measured
# twice: scalar-queue pieces spin up ~3us late and stall the PE).
X_PIECES = [(0, 1, "s"), (1, 1, "s"), (2, 2, "s"), (4, 2, "s"), (6, 2, "s"),
            (8, 4, "s"), (12, 4, "s"), (16, 4, "s"), (20, 4, "s"),
            (24, 4, "s"), (28, 2, "s"), (30, 2, "s")]
# R/Vt emission batches (start, len): even lengths (paired transposes),
# last batch tiny so the post-loop tail before softmax is short. (Coarser
# [(0,16),...] batches save ~2 R-bank reopens but inject a 1.4us R burst
# mid-stream — measured neutral; 8-tile batches keep the injections small.)
R_BATCHES = [(0, 8), (8, 8), (16, 8), (24, 6), (30, 2)]
N_WARMUP = 22             # PE p-state warmup matmuls during the DMA head

# O-phase tile order: even tiles (vT2 partitions 0-63) first, then odd
# tiles (partitions 64-127, needing the replicated P)
O_ORDER = list(range(0, NT, 2)) + list(range(1, NT, 2))


def _build():
    nc = bacc.Bacc("TRN2", target_bir_lowering=False, debug=False,
                   num_devices=N_CORES)
    # xt = [w3 | ident | mask | x]: w3 pre-packed partition-major
    # [128, ECH*M3], the PE-transpose identity and the (row-padded) causal
    # mask ride along too so NO gpsimd/SWDGE DMA machinery is needed; then
    # the tile-contiguous x pack where column HDR + (i*ECH + c)*128 + u
    # holds x[i*128+u, c*128+p]
    W3C = ECH * M3
    HDR = W3C + 128 + 64
    xt = nc.dram_tensor("xt", [128, HDR + NT * TW], BF16,
                        kind="ExternalInput").ap()
    # p-major output: column block n holds O tile O_ORDER[n] (host
    # un-permutes and casts to f32)
    out = nc.dram_tensor("out", [128, NT * D], BF16, kind="ExternalOutput").ap()

    with tile.TileContext(nc) as tc:
        with (
            tc.tile_pool(name="const", bufs=1) as constp,
            tc.tile_pool(name="xs", bufs=1) as xsp,
            tc.tile_pool(name="qkv", bufs=10) as qkvp,
            tc.tile_pool(name="vt", bufs=1) as vtp,
            tc.tile_pool(name="small", bufs=1) as smallp,
            tc.tile_pool(name="osb", bufs=8) as osbp,
            tc.tile_pool(name="ps_qkv", bufs=3, space="PSUM") as ps_qkv,
            tc.tile_pool(name="ps_r", bufs=1, space="PSUM") as ps_rp,
            tc.tile_pool(name="ps_o", bufs=4, space="PSUM") as ps_o,
        ):
            # piece 0 = [w3 | ident | mask | tile 0] in a single DMA; pieces
            # stream in consumption order on one queue
            xt_of = [None] * NT  # tile i -> (piece_ap, col0 within piece)
            w3_sb = ident_sb = mask_sb = None
            for t0, ntl, q in X_PIECES:
                pre = HDR if t0 == 0 else 0
                xs = xsp.tile([128, pre + ntl * TW], BF16, tag=f"xs{t0}",
                              name="xs")
                src0 = t0 * TW + (0 if t0 == 0 else HDR)
                eng = nc.sync if q == "s" else nc.scalar
                eng.dma_start(xs[:], xt[:, src0:src0 + pre + ntl * TW])
                if t0 == 0:
                    w3_sb = xs[:, 0:W3C]
                    ident_sb = xs[:, W3C:W3C + 128]
                    mask_sb = xs[0:64, W3C + 128:W3C + 192]
                for s in range(ntl):
                    xt_of[t0 + s] = (xs, pre + s * TW)

            # warmup scratch memset on gpsimd (pure engine op, no DMA queue)
            warm_sb = smallp.tile([128, M3], BF16, tag="warm")
            nc.gpsimd.memset(warm_sb[:], 0)

            # PE p-state warmup (results unread)
            for _ in range(N_WARMUP):
                pw = ps_qkv.tile([128, M3], F32, tag="qkv", name="pw")
                nc.tensor.matmul(pw[:], warm_sb[:, 0:128], warm_sb[:],
                                 start=True, stop=True)

            # persistent V^T: even tiles on partitions 0-63, odd on 64-127;
            # column block p holds pair (2p, 2p+1)
            vT2 = vtp.tile([128, (NT // 2) * 128], BF16)
            ps_R = ps_rp.tile([64, 64], F32, tag="r")  # persistent R accum

            qk_pairs = [None] * (NT // 2)         # [128, 256]: [Q0|K0|Q1|K1]
            v_pairs = [None] * (NT // 2)          # [128, 128]: [V0|V1]
            flush_at = {}  # qkv-tile index -> batch to emit when reached
            for b0, blen in R_BATCHES:
                nxt = b0 + blen  # first tile whose QKV burst hosts the flush
                flush_at[min(nxt + 1, NT - 1) if nxt < NT else NT] = (b0, blen)

            def emit_r(b0, blen):
                if b0 == 0:
                    # causal mask leads the first R burst (same accumulation
                    # group, no extra bank reopen): R = I64^T @ mask + ...
                    nc.tensor.matmul(ps_R[:], ident_sb[0:64, 0:64],
                                     mask_sb[:], start=True, stop=False)
                for i in range(b0, b0 + blen):
                    qk = qk_pairs[i // 2]
                    h = (i % 2) * 128
                    nc.tensor.matmul(
                        ps_R[:], qk[:, h + D:h + 2 * D], qk[:, h:h + D],
                        start=False, stop=(i == NT - 1),
                    )

            def emit_t(b0, blen):
                pvt = ps_o.tile([128, (blen // 2) * 128], BF16, tag="o",
                                name="pvt")
                for n in range(blen // 2):
                    nc.tensor.transpose(
                        pvt[:, n * 128:(n + 1) * 128],
                        v_pairs[b0 // 2 + n][:], ident_sb[:])
                nc.vector.tensor_copy(
                    vT2[:, (b0 // 2) * 128:((b0 + blen) // 2) * 128], pvt[:])

            for i in range(NT):
                xs, c0 = xt_of[i]
                pq = ps_qkv.tile([128, M3], F32, tag="qkv")
                for j in range(ECH):
                    nc.tensor.matmul(
                        pq[:], xs[:, c0 + j * 128:c0 + (j + 1) * 128],
                        w3_sb[:, j * M3:(j + 1) * M3],
                        start=(j == 0), stop=(j == ECH - 1),
                    )
                    # R matmuls at j==2, transposes at j==5: spreads the
                    # PSUM context switches across the tile's QKV burst
                    if j == 2 and i in flush_at:
                        emit_r(*flush_at[i])
                    if j == 5 and i in flush_at:
                        emit_t(*flush_at.pop(i))
                if i % 2 == 0:
                    qk_pairs[i // 2] = qkvp.tile([128, 256], BF16,
                                                 tag="qk_sb", name="qk2")
                    v_pairs[i // 2] = qkvp.tile([128, 128], BF16,
                                                tag="v_sb", name="v2")
                qk_dst = qk_pairs[i // 2][:, (i % 2) * 128:(i % 2) * 128 + 128]
                v_dst = v_pairs[i // 2][:, (i % 2) * D:(i % 2) * D + D]
                if i == NT - 1:
                    # last tile: split Q|K across both engines so the final
                    # R matmuls (softmax critical path) start sooner
                    nc.vector.tensor_copy(qk_dst[:, 0:D], pq[:, 0:D])
                    nc.scalar.activation(qk_dst[:, D:2 * D], pq[:, D:2 * D],
                                         AF.Copy)
                    nc.vector.tensor_copy(v_dst, pq[:, 128:192])
                elif i % 2 == 0:
                    nc.scalar.activation(qk_dst, pq[:, 0:128], AF.Copy)
                    nc.vector.tensor_copy(v_dst, pq[:, 128:192])
                else:
                    nc.vector.tensor_copy(qk_dst, pq[:, 0:128])
                    nc.scalar.activation(v_dst, pq[:, 128:192], AF.Copy)
            if NT in flush_at:
                b0, blen = flush_at.pop(NT)
                emit_r(b0, blen)
                emit_t(b0, blen)

            # ---- softmax on R (64x64), no max-shift (bounded scores) ----
            p_exp = smallp.tile([64, 64], F32)
            rowsum = smallp.tile([64, 1], F32)
            nc.scalar.activation(p_exp[:], ps_R[:], AF.Exp,
                                 bias=0.0, scale=1.0, accum_out=rowsum[:])
            rinv = smallp.tile([64, 1], F32)
            nc.vector.reciprocal(rinv[:], rowsum[:])
            p_r = smallp.tile([64, 64], BF16)
            nc.vector.tensor_scalar_mul(p_r[:], p_exp[:], rinv[:])

            # ---- O = V @ P : 4 groups of 8 chunks (evens, then odds). The
            # P-replication chain (P onto partitions 64-127 via two PE
            # transposes, for the odd-tile matmuls reading vT2's upper
            # half) interleaves with the even groups so its DVE copies
            # hide under PE work. Each group copies out in two halves on
            # ACT+DVE; group DMAs alternate sync/scalar queues. ----
            pt1 = ps_rp.tile([64, 64], BF16, tag="r", name="pt1")
            nc.tensor.transpose(pt1[:], p_r[:], ident_sb[0:64, 0:64])
            pts = smallp.tile([64, 64], BF16, name="pts")
            nc.vector.tensor_copy(pts[:], pt1[:])
            pt2 = None
            p2 = smallp.tile([128, 64], BF16, name="p2")
            out_q = [nc.sync, nc.scalar, nc.sync, nc.scalar]
            for grp in range(4):
                po = ps_o.tile([128, 8 * D], F32, tag="o")
                for k in range(8):
                    c = O_ORDER[grp * 8 + k]
                    hi, pr = c % 2, c // 2
                    lhsT = vT2[hi * 64:(hi + 1) * 64,
                               pr * 128:(pr + 1) * 128]
                    rhs = p_r[:] if hi == 0 else p2[64:128, :]
                    nc.tensor.matmul(po[:, k * D:(k + 1) * D], lhsT, rhs,
                                     start=True, stop=True)
                if grp == 0:
                    # second replication transpose rides between groups
                    pt2 = ps_rp.tile([128, 64], BF16, tag="r", name="pt2")
                    nc.tensor.transpose(pt2[64:128, :], pts[:],
                                        ident_sb[0:64, 0:64])
                    nc.vector.tensor_copy(p2[64:128, :], pt2[64:128, :])
                o_sb = osbp.tile([128, 8 * D], BF16, tag="o_sb")
                g0c = grp * 8 * D
                nc.scalar.activation(o_sb[:, 0:4 * D], po[:, 0:4 * D],
                                     AF.Copy)
                if grp == 3:
                    # last group: DMA each half right after its copy so the
                    # terminal config+DGE+sem chain starts earlier
                    nc.sync.dma_start(out[:, g0c:g0c + 4 * D],
                                      o_sb[:, 0:4 * D])
                    nc.vector.tensor_copy(o_sb[:, 4 * D:8 * D],
                                          po[:, 4 * D:8 * D])
                    nc.scalar.dma_start(out[:, g0c + 4 * D:g0c + 8 * D],
                                        o_sb[:, 4 * D:8 * D])
                else:
                    nc.vector.tensor_copy(o_sb[:, 4 * D:8 * D],
                                          po[:, 4 * D:8 * D])
                    out_q[grp].dma_start(out[:, g0c:g0c + 8 * D], o_sb[:])

    nc.compile()
    return nc


def _host_inputs(x, Wq, Wk, Wv):
    """Host-side prep: bf16 casts, tile-contiguous x pack, weight packing."""
    bf16 = ml_dtypes.bfloat16
    # fold the 1/sqrt(E) score scale into Wq (1/32 is exact in f32)
    w3f = np.concatenate(
        [np.asarray(Wq, np.float32) * (1.0 / 32.0),
         np.asarray(Wk, np.float32),
         np.asarray(Wv, np.float32)], axis=1).astype(bf16)  # [E, 192]
    # partition-major pack: w3h[p, c*192+m] = w3f[c*128+p, m]
    w3h = np.ascontiguousarray(
        w3f.reshape(ECH, 128, M3).transpose(1, 0, 2).reshape(128, ECH * M3))
    ident_h = np.eye(128, dtype=bf16)
    ii = np.arange(64)
    # additive mask: 0 where col >= row, -1e30 strictly below the diagonal
    mask_h = np.where(ii[None, :] >= ii[:, None], np.float32(0.0),
                      np.float32(-1e30)).astype(bf16)
    mask_pad = np.zeros((128, 64), dtype=bf16)
    mask_pad[0:64, :] = mask_h
    xb = np.asarray(x, np.float32).astype(bf16)  # (B, T, E)
    # xh[p, (i*ECH+c)*128+u] = x[i*128+u, c*128+p], prefixed by the header
    # [w3h | ident | mask]
    xtb = xb.reshape(B, NT, 128, ECH, 128).transpose(0, 4, 1, 3, 2) \
        .reshape(B, 128, NT * TW)
    hdr = np.concatenate([w3h, ident_h, mask_pad], axis=1)
    xtb = np.ascontiguousarray(
        np.concatenate([np.broadcast_to(hdr, (B,) + hdr.shape), xtb],
                       axis=2))
    return [{"xt": xtb[b]} for b in range(B)]


# inverse of O_ORDER: which output column block holds tile c
_O_POS = np.argsort(np.array(O_ORDER))


def kernel(x, Wq, Wk, Wv):
    global _COMPILED
    if _COMPILED is None:
        _COMPILED = _build()
    nc = _COMPILED

    in_maps = _host_inputs(x, Wq, Wk, Wv)
    res = run_bass_kernel_spmd(nc, in_maps, list(range(N_CORES)))
    # un-permute: device column block n holds O tile O_ORDER[n] with
    # out[p, n*64+d] = O[O_ORDER[n]*128+p, d]
    return np.stack([
        np.ascontiguousarray(
            np.asarray(res.results[b]["out"], dtype=np.float32)
            .reshape(128, NT, D)[:, _O_POS, :].transpose(1, 0, 2)
            .reshape(T, D))
        for b in range(B)
    ], axis=0)
